# revision 13
# baseline (speedup 1.0000x reference)
"""Trainium2 Bass kernel for the DMamba block (selective state-space / Mamba).

Sharding: tensor-parallel over d_inner across 8 NeuronCores (256 channels
each), d-major on chip so the selective scan maps onto tensor_tensor_scan
(one recurrence per partition along the free/time axis).

v2 vs baseline:
  - bf16 everywhere off the scan-state path: GEMM operands, B/C row
    broadcasts, collective payloads (scan state stays fp32 inside the DVE /
    Pool scan; only inputs are bf16-rounded).
  - silu fused into single ACT ops reading PSUM (conv branch + gate res).
  - B broadcast via DMA (like C) instead of PE matmuls; frees PE + PSUM.
  - scans split DVE/Pool (Pool's tensor_tensor_scan is cheaper), dBu and
    p=h*C on DVE in 2x bf16 mode, PSUM->SBUF copies on Pool/DVE.
  - per-batch AllReduce of x_dbl (bf16) overlapped with the other batch's
    in_proj/conv; per-batch ReduceScatter (bf16) with the b=0 one hidden
    behind the b=1 scan storm.
"""

import os
import sys
import time
from contextlib import ExitStack

import numpy as np

for _p in ("/opt/trn_rl_repo", "/root/.axon_site/_ro/trn_rl_repo"):
    if os.path.isdir(_p) and _p not in sys.path:
        sys.path.append(_p)

import ml_dtypes
import concourse.bacc as bacc
import concourse.mybir as mybir
import concourse.tile as tile
from concourse.bass_utils import run_bass_kernel_spmd

F32 = mybir.dt.float32
BF16 = mybir.dt.bfloat16
AF = mybir.ActivationFunctionType
OP = mybir.AluOpType
BF_NP = ml_dtypes.bfloat16

CFG_FULL = dict(B=2, L=1024, DM=1024, DI=2048, NST=16, RK=64, K4=4, NC=8)

# engine-assignment tunables for the scan storm (32 (n,dt) units per batch)
N_SCAN_DVE = 0      # scans on DVE per batch; rest on Pool
N_PC_POOL = 0       # p=h*C on Pool per batch; rest on DVE (bf16 2x)


def build_nc(cfg, no_cc=False, reps=1):
    B, L, DM, DI = cfg["B"], cfg["L"], cfg["DM"], cfg["DI"]
    NST, RK, K4, NC = cfg["NST"], cfg["RK"], cfg["K4"], cfg["NC"]
    DL = DI // NC                 # local d_inner channels (256)
    NDT = DL // 128               # local partition-tiles of d (2)
    TOK = B * L
    CH = 512                      # psum free-dim chunk
    LCH = L // CH                 # chunks per sequence (2)
    KC = DM // 128                # contraction chunks over d_model (8)
    XR = RK + 2 * NST             # x_dbl rows (96)
    PB = L // NC

    nc = bacc.Bacc("TRN2", target_bir_lowering=False, debug=False, num_devices=NC)

    def din(name, shape, dt=F32):
        return nc.dram_tensor(name, shape, dt, kind="ExternalInput").ap()

    xT = din("xT", [DM, TOK], BF16)
    w_in_T = din("w_in_T", [DM, 2 * DL], BF16)
    convdiag = din("convdiag", [NDT * K4 * 128, 128], BF16)
    w_x_T = din("w_x_T", [DL, XR], BF16)
    w_dt_T = din("w_dt_T", [RK, DL], BF16)
    b_dt_col = din("b_dt_col", [DL, 1])
    a_col = din("a_col", [DL, NST])
    d_col = din("d_col", [DL, 1])
    conv_b_col = din("conv_b_col", [DL, 1])
    w_out_T = din("w_out_T", [DL, DM], BF16)
    id128 = din("id128", [128, 128], BF16)

    out_ext = nc.dram_tensor("out", [TOK // NC, DM], BF16, kind="ExternalOutput").ap()

    shared = "Shared" if NC > 4 else "Local"
    # per-batch row blocks so collective payloads are contiguous
    xdbl_part = nc.dram_tensor("xdbl_part", [B * XR, L], BF16).ap()
    xdbl_full = nc.dram_tensor("xdbl_full", [B * XR, L], BF16, addr_space=shared).ap()
    out_part = nc.dram_tensor("out_part", [TOK, DM], BF16).ap()
    out_rs = nc.dram_tensor("out_rs", [TOK // NC, DM], BF16).ap()

    groups = [list(range(NC))]

    with tile.TileContext(nc) as tc, ExitStack() as ctx:
        consts = ctx.enter_context(tc.tile_pool(name="consts", bufs=1))
        big = ctx.enter_context(tc.tile_pool(name="big", bufs=1))
        work = ctx.enter_context(tc.tile_pool(name="work", bufs=2))
        mm = ctx.enter_context(tc.tile_pool(name="mm", bufs=3, space="PSUM"))

        # ---- constants ----
        a_t, d_t, bdt_t, cb_t, wout_t, wx_t, cdg_t = [], [], [], [], [], [], []
        for dt in range(NDT):
            t = consts.tile([128, NST], F32, name=f"a{dt}")
            nc.gpsimd.dma_start(out=t[:], in_=a_col[dt * 128:(dt + 1) * 128, :])
            a_t.append(t)
            t = consts.tile([128, 1], F32, name=f"d{dt}")
            nc.gpsimd.dma_start(out=t[:], in_=d_col[dt * 128:(dt + 1) * 128, :])
            d_t.append(t)
            t = consts.tile([128, 1], F32, name=f"bdt{dt}")
            nc.gpsimd.dma_start(out=t[:], in_=b_dt_col[dt * 128:(dt + 1) * 128, :])
            bdt_t.append(t)
            t = consts.tile([128, 1], F32, name=f"cb{dt}")
            nc.gpsimd.dma_start(out=t[:], in_=conv_b_col[dt * 128:(dt + 1) * 128, :])
            cb_t.append(t)
            t = consts.tile([128, XR], BF16, name=f"wx{dt}")
            nc.gpsimd.dma_start(out=t[:], in_=w_x_T[dt * 128:(dt + 1) * 128, :])
            wx_t.append(t)
            row = []
            for i in range(K4):
                t = consts.tile([128, 128], BF16, name=f"cd{dt}_{i}")
                off = (dt * K4 + i) * 128
                nc.gpsimd.dma_start(out=t[:], in_=convdiag[off:off + 128, :])
                row.append(t)
            cdg_t.append(row)
        wdt_t = consts.tile([RK, DL], BF16, name="wdt")
        nc.gpsimd.dma_start(out=wdt_t[:], in_=w_dt_T[:])
        id_t = consts.tile([128, 128], BF16, name="id128")
        nc.gpsimd.dma_start(out=id_t[:], in_=id128[:])
        win_t = []
        for k in range(KC):
            t = consts.tile([128, 2 * DL], BF16, name=f"win{k}")
            nc.scalar.dma_start(out=t[:], in_=w_in_T[k * 128:(k + 1) * 128, :])
            win_t.append(t)
        for dt in range(NDT):
            t = consts.tile([128, DM], BF16, name=f"wo{dt}")
            nc.scalar.dma_start(out=t[:], in_=w_out_T[dt * 128:(dt + 1) * 128, :])
            wout_t.append(t)

        for rep in range(reps):
            # ---- persistent per-rep intermediates ----
            xc_t = [big.tile([128, TOK], BF16, name=f"xc{dt}") for dt in range(NDT)]
            sres_t = [big.tile([128, TOK], BF16, name=f"sres{dt}") for dt in range(NDT)]
            delta_t = [big.tile([128, TOK], F32, name=f"delta{dt}") for dt in range(NDT)]
            dx_t = [big.tile([128, TOK], BF16, name=f"dx{dt}") for dt in range(NDT)]
            xz_pad = [[big.tile([128, L + K4 - 1], BF16, name=f"xzp{b}_{dt}")
                       for dt in range(NDT)] for b in range(B)]
            xdbl_sb = [big.tile([XR, L], BF16, name=f"xdbl{b}") for b in range(B)]
            dlt_sb = [big.tile([RK, L], BF16, name=f"dlt{b}") for b in range(B)]

            copy_engs = [nc.gpsimd, nc.vector]

            # ================= PHASE 1: per-batch GEMM front-end =========
            for b in range(B):
                for dt in range(NDT):
                    nc.gpsimd.memset(xz_pad[b][dt][:, 0:K4 - 1], 0.0)
                # in_proj
                for tch in range(LCH):
                    off = tch * CH
                    xtc = []
                    for k in range(KC):
                        t = work.tile([128, CH], BF16, name=f"xtc{k}", bufs=2)
                        nc.scalar.dma_start(
                            out=t[:],
                            in_=xT[k * 128:(k + 1) * 128, b * L + off:b * L + off + CH])
                        xtc.append(t)
                    for m in range(2 * NDT):
                        ps = mm.tile([128, CH], F32, name="ps")
                        for k in range(KC):
                            nc.tensor.matmul(
                                ps[:], win_t[k][:, m * 128:(m + 1) * 128],
                                xtc[k][:],
                                start=(k == 0), stop=(k == KC - 1))
                        if m < NDT:
                            eng = copy_engs[(tch * NDT + m) % 2]
                            eng.tensor_copy(
                                xz_pad[b][m][:, K4 - 1 + off:K4 - 1 + off + CH],
                                ps[:])
                        else:
                            dt = m - NDT
                            sg = work.tile([128, CH], BF16, name="sg", bufs=4)
                            nc.scalar.activation(sg[:], ps[:], AF.Sigmoid)
                            nc.gpsimd.tensor_tensor(
                                sres_t[dt][:, b * L + off:b * L + off + CH],
                                ps[:], sg[:], OP.mult)
                # causal depthwise conv + silu (silu(x+cb) = (x+cb)*sigmoid(x+cb))
                for dt in range(NDT):
                    for lc in range(LCH):
                        ps = mm.tile([128, CH], F32, name="ps")
                        for i in range(K4):
                            nc.tensor.matmul(
                                ps[:], cdg_t[dt][i],
                                xz_pad[b][dt][:, lc * CH + i:lc * CH + i + CH],
                                start=(i == 0), stop=(i == K4 - 1))
                        sg = work.tile([128, CH], BF16, name="sg", bufs=4)
                        nc.scalar.activation(sg[:], ps[:], AF.Sigmoid,
                                             bias=cb_t[dt][:])
                        nc.vector.scalar_tensor_tensor(
                            xc_t[dt][:, b * L + lc * CH:b * L + (lc + 1) * CH],
                            ps[:], cb_t[dt][:], sg[:], OP.add, OP.mult)
                # x_proj partials
                for tch in range(LCH):
                    ps = mm.tile([XR, CH], F32, name="ps")
                    for dt in range(NDT):
                        nc.tensor.matmul(
                            ps[:], wx_t[dt][:],
                            xc_t[dt][:, b * L + tch * CH:b * L + (tch + 1) * CH],
                            start=(dt == 0), stop=(dt == NDT - 1))
                    copy_engs[tch % 2].tensor_copy(
                        xdbl_sb[b][:, tch * CH:(tch + 1) * CH], ps[:])
                nc.sync.dma_start(out=xdbl_part[b * XR:(b + 1) * XR, :],
                                  in_=xdbl_sb[b][:])
                if no_cc:
                    nc.sync.dma_start(out=xdbl_full[b * XR:(b + 1) * XR, :],
                                      in_=xdbl_part[b * XR:(b + 1) * XR, :])
                else:
                    nc.gpsimd.collective_compute(
                        "AllReduce", OP.add, replica_groups=groups,
                        ins=[xdbl_part[b * XR:(b + 1) * XR, :]],
                        outs=[xdbl_full[b * XR:(b + 1) * XR, :]])

            # ================= PHASE 2: per-batch scan + out =============
            rctx = ExitStack()
            bc = rctx.enter_context(tc.tile_pool(name=f"bc{rep}", bufs=1))
            scanp = rctx.enter_context(tc.tile_pool(name=f"scanp{rep}", bufs=3))
            acc = rctx.enter_context(
                tc.tile_pool(name=f"acc{rep}", bufs=1, space="PSUM"))
            osb_pool = rctx.enter_context(tc.tile_pool(name=f"osb{rep}", bufs=4))

            for b in range(B):
                base = b * XR
                # dlt rows to SBUF for dt_proj
                nc.sync.dma_start(out=dlt_sb[b][:],
                                  in_=xdbl_full[base:base + RK, :])
                # B/C row broadcasts (bf16), round-robin DMA queues
                brc, crc = [], []
                qs = [nc.sync, nc.sync]
                for n in range(NST):
                    t = bc.tile([128, L], BF16, name="brc", bufs=10)
                    qs[n % 2].dma_start(
                        out=t[:],
                        in_=xdbl_full[base + RK + n:base + RK + n + 1,
                                      :].to_broadcast((128, L)))
                    brc.append(t)
                    t = bc.tile([128, L], BF16, name="crc", bufs=10)
                    qs[(n + 1) % 2].dma_start(
                        out=t[:],
                        in_=xdbl_full[base + RK + NST + n:base + RK + NST + n + 1,
                                      :].to_broadcast((128, L)))
                    crc.append(t)
                # delta = softplus(W_dt @ dlt + b_dt)  (exp then ln(1+e));
                # all Exp ops batched before all Ln ops to minimize
                # activation-table reloads.
                sp_es = []
                for tch in range(LCH):
                    for dt in range(NDT):
                        ps = mm.tile([128, CH], F32, name="ps")
                        nc.tensor.matmul(
                            ps[:], wdt_t[:, dt * 128:(dt + 1) * 128],
                            dlt_sb[b][:, tch * CH:(tch + 1) * CH],
                            start=True, stop=True)
                        e = work.tile([128, CH], F32, name="sptmp", bufs=4)
                        nc.scalar.activation(e[:], ps[:], AF.Exp, bias=bdt_t[dt][:])
                        sp_es.append((tch, dt, e))
                for tch, dt, e in sp_es:
                    nc.scalar.activation(
                        delta_t[dt][:, b * L + tch * CH:b * L + (tch + 1) * CH],
                        e[:], AF.Ln, bias=1.0)
                # dx = delta * xc (bf16 2x)
                for dt in range(NDT):
                    sl = slice(b * L, (b + 1) * L)
                    nc.vector.tensor_tensor(
                        dx_t[dt][:, sl], delta_t[dt][:, sl], xc_t[dt][:, sl],
                        OP.mult)

                # ---- scan storm ----
                y_ps = [acc.tile([128, L], F32, name=f"y{dt}") for dt in range(NDT)]
                for n in range(NST):
                    for dt in range(NDT):
                        i = n * NDT + dt
                        sl = slice(b * L, (b + 1) * L)
                        dA = scanp.tile([128, L], F32, name="dA")
                        nc.scalar.activation(
                            dA[:], delta_t[dt][:, sl], AF.Exp,
                            scale=a_t[dt][:, n:n + 1])
                        dBu = scanp.tile([128, L], BF16, name="dBu")
                        nc.vector.tensor_tensor(
                            dBu[:], dx_t[dt][:, sl], brc[n][:], OP.mult)
                        h = scanp.tile([128, L], BF16, name="h")
                        scan_eng = nc.vector if i < N_SCAN_DVE * NDT else nc.gpsimd
                        scan_eng.tensor_tensor_scan(
                            h[:], dA[:], dBu[:], 0.0, OP.mult, OP.add)
                        p = scanp.tile([128, L], BF16, name="p")
                        pc_eng = nc.gpsimd if i < N_PC_POOL else nc.vector
                        pc_eng.tensor_tensor(p[:], h[:], crc[n][:], OP.mult)
                        for lc in range(LCH):
                            nc.tensor.matmul(
                                y_ps[dt][:, lc * CH:(lc + 1) * CH], id_t[:],
                                p[:, lc * CH:(lc + 1) * CH],
                                start=(n == 0), stop=(n == NST - 1),
                                skip_group_check=True)
                # gating: yg = (y + D*xc) * sres
                yg = []
                for dt in range(NDT):
                    sl = slice(b * L, (b + 1) * L)
                    t1 = work.tile([128, L], BF16, name="t1")
                    nc.vector.scalar_tensor_tensor(
                        t1[:], xc_t[dt][:, sl], d_t[dt][:], y_ps[dt][:],
                        OP.mult, OP.add)
                    ygt = scanp.tile([128, L], BF16, name=f"yg{dt}")
                    nc.vector.tensor_tensor(ygt[:], t1[:], sres_t[dt][:, sl],
                                            OP.mult)
                    yg.append(ygt)
                # out_proj
                for m in range(L // 128):
                    ps = mm.tile([128, CH], F32, name="ps")
                    ps2 = mm.tile([128, CH], F32, name="ps")
                    for dt in range(NDT):
                        nc.tensor.matmul(
                            ps[:], yg[dt][:, m * 128:(m + 1) * 128],
                            wout_t[dt][:, 0:CH],
                            start=(dt == 0), stop=(dt == NDT - 1))
                        nc.tensor.matmul(
                            ps2[:], yg[dt][:, m * 128:(m + 1) * 128],
                            wout_t[dt][:, CH:2 * CH],
                            start=(dt == 0), stop=(dt == NDT - 1))
                    oc = osb_pool.tile([128, DM], BF16, name="oc", bufs=4)
                    e1, e2 = (nc.vector, nc.gpsimd) if m % 2 else (nc.gpsimd, nc.vector)
                    e1.tensor_copy(oc[:, 0:CH], ps[:])
                    e2.tensor_copy(oc[:, CH:2 * CH], ps2[:])
                    nc.sync.dma_start(
                        out=out_part[b * L + m * 128:b * L + (m + 1) * 128, :],
                        in_=oc[:])
            # ReduceScatters emitted after both storms so they don't
            # head-of-line-block the pool queue's scan work.
            for b in range(B):
                if no_cc:
                    nc.sync.dma_start(
                        out=out_rs[b * PB:(b + 1) * PB, :],
                        in_=out_part[b * L:b * L + PB, :])
                else:
                    nc.gpsimd.collective_compute(
                        "ReduceScatter", OP.add, replica_groups=groups,
                        ins=[out_part[b * L:(b + 1) * L, :]],
                        outs=[out_rs[b * PB:(b + 1) * PB, :]])
                nc.sync.dma_start(out=out_ext[b * PB:(b + 1) * PB, :],
                                  in_=out_rs[b * PB:(b + 1) * PB, :])
            rctx.close()

    nc.compile()
    return nc


def prep_inputs(inputs, cfg):
    """Host-side sharding/transposition. Returns per-core input maps."""
    B, L, DM, DI = cfg["B"], cfg["L"], cfg["DM"], cfg["DI"]
    NST, RK, K4, NC = cfg["NST"], cfg["RK"], cfg["K4"], cfg["NC"]
    DL = DI // NC
    NDT = DL // 128
    TOK = B * L

    x = np.asarray(inputs["x"], np.float32)
    W_in = np.asarray(inputs["W_in"], np.float32)
    conv_w = np.asarray(inputs["conv_w"], np.float32)
    conv_b = np.asarray(inputs["conv_b"], np.float32)
    W_x = np.asarray(inputs["W_x"], np.float32)
    W_dt = np.asarray(inputs["W_dt"], np.float32)
    b_dt = np.asarray(inputs["b_dt"], np.float32)
    A_log = np.asarray(inputs["A_log"], np.float32)
    D = np.asarray(inputs["D"], np.float32)
    W_out = np.asarray(inputs["W_out"], np.float32)

    xT = np.ascontiguousarray(x.reshape(TOK, DM).T).astype(BF_NP)
    id128 = np.eye(128, dtype=BF_NP)
    in_maps = []
    for c in range(NC):
        sl = slice(c * DL, (c + 1) * DL)
        w_in_sel = np.concatenate([W_in[sl], W_in[DI + c * DL:DI + (c + 1) * DL]], 0)
        cd = np.zeros((NDT * K4 * 128, 128), np.float32)
        for dt in range(NDT):
            for i in range(K4):
                off = (dt * K4 + i) * 128
                np.fill_diagonal(cd[off:off + 128],
                                 conv_w[c * DL + dt * 128:c * DL + (dt + 1) * 128, i])
        in_maps.append({
            "xT": xT,
            "w_in_T": np.ascontiguousarray(w_in_sel.T).astype(BF_NP),
            "convdiag": cd.astype(BF_NP),
            "w_x_T": np.ascontiguousarray(W_x[:, sl].T).astype(BF_NP),
            "w_dt_T": np.ascontiguousarray(W_dt[sl].T).astype(BF_NP),
            "b_dt_col": np.ascontiguousarray(b_dt[sl])[:, None],
            "a_col": np.ascontiguousarray(-np.exp(A_log[sl])),
            "d_col": np.ascontiguousarray(D[sl])[:, None],
            "conv_b_col": np.ascontiguousarray(conv_b[sl])[:, None],
            "w_out_T": np.ascontiguousarray(W_out[:, sl].T).astype(BF_NP),
            "id128": id128,
        })
    return in_maps


_NC_CACHE = {}


def _get_nc(cfg):
    key = tuple(sorted(cfg.items()))
    if key not in _NC_CACHE:
        _NC_CACHE[key] = build_nc(cfg)
    return _NC_CACHE[key]


def run_cfg(inputs, cfg, time_iters=0):
    nc = _get_nc(cfg)
    NC = cfg["NC"]
    in_maps = prep_inputs(inputs, cfg)
    res = run_bass_kernel_spmd(nc, in_maps, list(range(NC)))
    wall_ns = None
    if time_iters:
        times = []
        for _ in range(time_iters):
            t0 = time.perf_counter()
            res = run_bass_kernel_spmd(nc, in_maps, list(range(NC)))
            times.append(time.perf_counter() - t0)
        wall_ns = min(times) * 1e9
    B, L, DM = cfg["B"], cfg["L"], cfg["DM"]
    PB = L // NC
    parts = []
    for b in range(B):
        for c in range(NC):
            parts.append(np.asarray(res.results[c]["out"][b * PB:(b + 1) * PB],
                                    np.float32))
    out = np.concatenate(parts, 0)
    return out.reshape(B, L, DM), wall_ns


def kernel(**inputs):
    out, _ = run_cfg(inputs, CFG_FULL)
    return out.astype(np.float32)


# revision 16
# speedup vs baseline: 1.0329x; 1.0329x over previous
"""Trainium2 Bass kernel for the DMamba block (selective state-space / Mamba).

Sharding: tensor-parallel over d_inner across 8 NeuronCores (256 channels
each), d-major on chip so the selective scan maps onto tensor_tensor_scan
(one recurrence per partition along the free/time axis).

v2 vs baseline:
  - bf16 everywhere off the scan-state path: GEMM operands, B/C row
    broadcasts, collective payloads (scan state stays fp32 inside the DVE /
    Pool scan; only inputs are bf16-rounded).
  - silu fused into single ACT ops reading PSUM (conv branch + gate res).
  - B broadcast via DMA (like C) instead of PE matmuls; frees PE + PSUM.
  - scans split DVE/Pool (Pool's tensor_tensor_scan is cheaper), dBu and
    p=h*C on DVE in 2x bf16 mode, PSUM->SBUF copies on Pool/DVE.
  - per-batch AllReduce of x_dbl (bf16) overlapped with the other batch's
    in_proj/conv; per-batch ReduceScatter (bf16) with the b=0 one hidden
    behind the b=1 scan storm.
"""

import os
import sys
import time
from contextlib import ExitStack

import numpy as np

for _p in ("/opt/trn_rl_repo", "/root/.axon_site/_ro/trn_rl_repo"):
    if os.path.isdir(_p) and _p not in sys.path:
        sys.path.append(_p)

import ml_dtypes
import concourse.bacc as bacc
import concourse.mybir as mybir
import concourse.tile as tile
from concourse.bass_utils import run_bass_kernel_spmd

F32 = mybir.dt.float32
BF16 = mybir.dt.bfloat16
AF = mybir.ActivationFunctionType
OP = mybir.AluOpType
BF_NP = ml_dtypes.bfloat16

CFG_FULL = dict(B=2, L=1024, DM=1024, DI=2048, NST=16, RK=64, K4=4, NC=8)

# engine-assignment tunables for the scan storm (32 (n,dt) units per batch)
N_SCAN_DVE = 0      # scans on DVE per batch; rest on Pool
N_PC_POOL = 0       # p=h*C on Pool per batch; rest on DVE (bf16 2x)


def build_nc(cfg, no_cc=False, reps=1):
    B, L, DM, DI = cfg["B"], cfg["L"], cfg["DM"], cfg["DI"]
    NST, RK, K4, NC = cfg["NST"], cfg["RK"], cfg["K4"], cfg["NC"]
    DL = DI // NC                 # local d_inner channels (256)
    NDT = DL // 128               # local partition-tiles of d (2)
    TOK = B * L
    CH = 512                      # psum free-dim chunk
    LCH = L // CH                 # chunks per sequence (2)
    KC = DM // 128                # contraction chunks over d_model (8)
    XR = RK + 2 * NST             # x_dbl rows (96)
    PB = L // NC

    nc = bacc.Bacc("TRN2", target_bir_lowering=False, debug=False, num_devices=NC)

    def din(name, shape, dt=F32):
        return nc.dram_tensor(name, shape, dt, kind="ExternalInput").ap()

    xT = din("xT", [DM, TOK], BF16)
    w_in_T = din("w_in_T", [DM, 2 * DL], BF16)
    convdiag = din("convdiag", [NDT * K4 * 128, 128], BF16)
    w_x_T = din("w_x_T", [DL, XR], BF16)
    w_dt_T = din("w_dt_T", [RK, DL], BF16)
    b_dt_col = din("b_dt_col", [DL, 1])
    a_col = din("a_col", [DL, NST])
    d_col = din("d_col", [DL, 1])
    conv_b_col = din("conv_b_col", [DL, 1])
    w_out_T = din("w_out_T", [DL, DM], BF16)
    id128 = din("id128", [128, 128], BF16)

    out_ext = nc.dram_tensor("out", [TOK // NC, DM], BF16, kind="ExternalOutput").ap()

    shared = "Shared" if NC > 4 else "Local"
    # per-batch row blocks so collective payloads are contiguous
    xdbl_part = nc.dram_tensor("xdbl_part", [B * XR, L], BF16).ap()
    xdbl_full = nc.dram_tensor("xdbl_full", [B * XR, L], BF16, addr_space=shared).ap()
    out_part = nc.dram_tensor("out_part", [TOK, DM], BF16).ap()
    out_rs = nc.dram_tensor("out_rs", [TOK // NC, DM], BF16).ap()

    groups = [list(range(NC))]

    with tile.TileContext(nc) as tc, ExitStack() as ctx:
        consts = ctx.enter_context(tc.tile_pool(name="consts", bufs=1))
        big = ctx.enter_context(tc.tile_pool(name="big", bufs=1))
        work = ctx.enter_context(tc.tile_pool(name="work", bufs=2))
        mm = ctx.enter_context(tc.tile_pool(name="mm", bufs=3, space="PSUM"))

        # ---- constants ----
        a_t, d_t, bdt_t, cb_t, wout_t, wx_t, cdg_t = [], [], [], [], [], [], []
        for dt in range(NDT):
            t = consts.tile([128, NST], F32, name=f"a{dt}")
            nc.gpsimd.dma_start(out=t[:], in_=a_col[dt * 128:(dt + 1) * 128, :])
            a_t.append(t)
            t = consts.tile([128, 1], F32, name=f"d{dt}")
            nc.gpsimd.dma_start(out=t[:], in_=d_col[dt * 128:(dt + 1) * 128, :])
            d_t.append(t)
            t = consts.tile([128, 1], F32, name=f"bdt{dt}")
            nc.gpsimd.dma_start(out=t[:], in_=b_dt_col[dt * 128:(dt + 1) * 128, :])
            bdt_t.append(t)
            t = consts.tile([128, 1], F32, name=f"cb{dt}")
            nc.gpsimd.dma_start(out=t[:], in_=conv_b_col[dt * 128:(dt + 1) * 128, :])
            cb_t.append(t)
            t = consts.tile([128, XR], BF16, name=f"wx{dt}")
            nc.gpsimd.dma_start(out=t[:], in_=w_x_T[dt * 128:(dt + 1) * 128, :])
            wx_t.append(t)
            row = []
            for i in range(K4):
                t = consts.tile([128, 128], BF16, name=f"cd{dt}_{i}")
                off = (dt * K4 + i) * 128
                nc.gpsimd.dma_start(out=t[:], in_=convdiag[off:off + 128, :])
                row.append(t)
            cdg_t.append(row)
        wdt_t = consts.tile([RK, DL], BF16, name="wdt")
        nc.gpsimd.dma_start(out=wdt_t[:], in_=w_dt_T[:])
        id_t = consts.tile([128, 128], BF16, name="id128")
        nc.gpsimd.dma_start(out=id_t[:], in_=id128[:])
        win_t = []
        for k in range(KC):
            t = consts.tile([128, 2 * DL], BF16, name=f"win{k}")
            nc.scalar.dma_start(out=t[:], in_=w_in_T[k * 128:(k + 1) * 128, :])
            win_t.append(t)
        for dt in range(NDT):
            t = consts.tile([128, DM], BF16, name=f"wo{dt}")
            nc.scalar.dma_start(out=t[:], in_=w_out_T[dt * 128:(dt + 1) * 128, :])
            wout_t.append(t)

        for rep in range(reps):
            # ---- persistent per-rep intermediates ----
            xc_t = [big.tile([128, TOK], BF16, name=f"xc{dt}") for dt in range(NDT)]
            sres_t = [big.tile([128, TOK], BF16, name=f"sres{dt}") for dt in range(NDT)]
            delta_t = [big.tile([128, TOK], F32, name=f"delta{dt}") for dt in range(NDT)]
            dx_t = [big.tile([128, TOK], BF16, name=f"dx{dt}") for dt in range(NDT)]
            xz_pad = [[big.tile([128, L + K4 - 1], BF16, name=f"xzp{b}_{dt}")
                       for dt in range(NDT)] for b in range(B)]
            xdbl_sb = [big.tile([XR, L], BF16, name=f"xdbl{b}") for b in range(B)]
            dlt_sb = [big.tile([RK, L], BF16, name=f"dlt{b}") for b in range(B)]

            copy_engs = [nc.gpsimd, nc.vector]

            # ================= PHASE 1: per-batch GEMM front-end =========
            for b in range(B):
                for dt in range(NDT):
                    nc.gpsimd.memset(xz_pad[b][dt][:, 0:K4 - 1], 0.0)
                # in_proj
                for tch in range(LCH):
                    off = tch * CH
                    xtc = []
                    for k in range(KC):
                        t = work.tile([128, CH], BF16, name=f"xtc{k}", bufs=2)
                        nc.sync.dma_start(
                            out=t[:],
                            in_=xT[k * 128:(k + 1) * 128, b * L + off:b * L + off + CH])
                        xtc.append(t)
                    for m in range(2 * NDT):
                        ps = mm.tile([128, CH], F32, name="ps")
                        for k in range(KC):
                            nc.tensor.matmul(
                                ps[:], win_t[k][:, m * 128:(m + 1) * 128],
                                xtc[k][:],
                                start=(k == 0), stop=(k == KC - 1))
                        if m < NDT:
                            eng = copy_engs[(tch * NDT + m) % 2]
                            eng.tensor_copy(
                                xz_pad[b][m][:, K4 - 1 + off:K4 - 1 + off + CH],
                                ps[:])
                        else:
                            dt = m - NDT
                            sg = work.tile([128, CH], BF16, name="sg", bufs=4)
                            nc.scalar.activation(sg[:], ps[:], AF.Sigmoid)
                            nc.gpsimd.tensor_tensor(
                                sres_t[dt][:, b * L + off:b * L + off + CH],
                                ps[:], sg[:], OP.mult)
                # causal depthwise conv + silu (silu(x+cb) = (x+cb)*sigmoid(x+cb))
                for dt in range(NDT):
                    for lc in range(LCH):
                        ps = mm.tile([128, CH], F32, name="ps")
                        for i in range(K4):
                            nc.tensor.matmul(
                                ps[:], cdg_t[dt][i],
                                xz_pad[b][dt][:, lc * CH + i:lc * CH + i + CH],
                                start=(i == 0), stop=(i == K4 - 1))
                        sg = work.tile([128, CH], BF16, name="sg", bufs=4)
                        nc.scalar.activation(sg[:], ps[:], AF.Sigmoid,
                                             bias=cb_t[dt][:])
                        nc.vector.scalar_tensor_tensor(
                            xc_t[dt][:, b * L + lc * CH:b * L + (lc + 1) * CH],
                            ps[:], cb_t[dt][:], sg[:], OP.add, OP.mult)
                # x_proj partials
                for tch in range(LCH):
                    ps = mm.tile([XR, CH], F32, name="ps")
                    for dt in range(NDT):
                        nc.tensor.matmul(
                            ps[:], wx_t[dt][:],
                            xc_t[dt][:, b * L + tch * CH:b * L + (tch + 1) * CH],
                            start=(dt == 0), stop=(dt == NDT - 1))
                    copy_engs[tch % 2].tensor_copy(
                        xdbl_sb[b][:, tch * CH:(tch + 1) * CH], ps[:])
                nc.sync.dma_start(out=xdbl_part[b * XR:(b + 1) * XR, :],
                                  in_=xdbl_sb[b][:])
                if no_cc:
                    nc.sync.dma_start(out=xdbl_full[b * XR:(b + 1) * XR, :],
                                      in_=xdbl_part[b * XR:(b + 1) * XR, :])
                else:
                    nc.gpsimd.collective_compute(
                        "AllReduce", OP.add, replica_groups=groups,
                        ins=[xdbl_part[b * XR:(b + 1) * XR, :]],
                        outs=[xdbl_full[b * XR:(b + 1) * XR, :]])

            # ================= PHASE 2: per-batch scan + out =============
            rctx = ExitStack()
            bc = rctx.enter_context(tc.tile_pool(name=f"bc{rep}", bufs=1))
            scanp = rctx.enter_context(tc.tile_pool(name=f"scanp{rep}", bufs=3))
            acc = rctx.enter_context(
                tc.tile_pool(name=f"acc{rep}", bufs=1, space="PSUM"))
            osb_pool = rctx.enter_context(tc.tile_pool(name=f"osb{rep}", bufs=4))

            for b in range(B):
                base = b * XR
                # dlt rows to SBUF for dt_proj
                nc.sync.dma_start(out=dlt_sb[b][:],
                                  in_=xdbl_full[base:base + RK, :])
                # B/C row broadcasts (bf16), round-robin DMA queues
                brc, crc = [], []
                qs = [nc.sync, nc.sync]
                for n in range(NST):
                    t = bc.tile([128, L], BF16, name="brc", bufs=7)
                    qs[n % 2].dma_start(
                        out=t[:],
                        in_=xdbl_full[base + RK + n:base + RK + n + 1,
                                      :].to_broadcast((128, L)))
                    brc.append(t)
                    t = bc.tile([128, L], BF16, name="crc", bufs=7)
                    qs[(n + 1) % 2].dma_start(
                        out=t[:],
                        in_=xdbl_full[base + RK + NST + n:base + RK + NST + n + 1,
                                      :].to_broadcast((128, L)))
                    crc.append(t)
                # delta = softplus(W_dt @ dlt + b_dt)  (exp then ln(1+e));
                # all Exp ops batched before all Ln ops to minimize
                # activation-table reloads.
                sp_es = []
                for tch in range(LCH):
                    for dt in range(NDT):
                        ps = mm.tile([128, CH], F32, name="ps")
                        nc.tensor.matmul(
                            ps[:], wdt_t[:, dt * 128:(dt + 1) * 128],
                            dlt_sb[b][:, tch * CH:(tch + 1) * CH],
                            start=True, stop=True)
                        e = work.tile([128, CH], F32, name="sptmp", bufs=4)
                        nc.scalar.activation(e[:], ps[:], AF.Exp, bias=bdt_t[dt][:])
                        sp_es.append((tch, dt, e))
                for tch, dt, e in sp_es:
                    nc.scalar.activation(
                        delta_t[dt][:, b * L + tch * CH:b * L + (tch + 1) * CH],
                        e[:], AF.Ln, bias=1.0)
                # dx = delta * xc (bf16 2x)
                for dt in range(NDT):
                    sl = slice(b * L, (b + 1) * L)
                    nc.vector.tensor_tensor(
                        dx_t[dt][:, sl], delta_t[dt][:, sl], xc_t[dt][:, sl],
                        OP.mult)

                # ---- scan storm ----
                # all dBu emitted first so DVE never head-blocks on pool scans
                y_ps = [acc.tile([128, L], F32, name=f"y{dt}") for dt in range(NDT)]
                sl = slice(b * L, (b + 1) * L)
                NU = NST * NDT

                def emit_dbu(i):
                    n, dt = divmod(i, NDT)
                    dBu = scanp.tile([128, L], BF16, name="dBu", bufs=18)
                    nc.vector.tensor_tensor(
                        dBu[:], dx_t[dt][:, sl], brc[n][:], OP.mult)
                    return dBu

                dBus = [emit_dbu(i) for i in range(NU // 2)]
                for n in range(NST):
                    for dt in range(NDT):
                        i = n * NDT + dt
                        if i + NU // 2 < NU:
                            dBus.append(emit_dbu(i + NU // 2))
                        dA = scanp.tile([128, L], F32, name="dA")
                        nc.scalar.activation(
                            dA[:], delta_t[dt][:, sl], AF.Exp,
                            scale=a_t[dt][:, n:n + 1])
                        h = scanp.tile([128, L], BF16, name="h")
                        scan_eng = nc.vector if i < N_SCAN_DVE * NDT else nc.gpsimd
                        scan_eng.tensor_tensor_scan(
                            h[:], dA[:], dBus[i][:], 0.0, OP.mult, OP.add)
                        p = scanp.tile([128, L], BF16, name="p", bufs=2)
                        pc_eng = nc.gpsimd if i < N_PC_POOL else nc.vector
                        pc_eng.tensor_tensor(p[:], h[:], crc[n][:], OP.mult)
                        for lc in range(LCH):
                            nc.tensor.matmul(
                                y_ps[dt][:, lc * CH:(lc + 1) * CH], id_t[:],
                                p[:, lc * CH:(lc + 1) * CH],
                                start=(n == 0), stop=(n == NST - 1),
                                skip_group_check=True)
                # gating: yg = (y + D*xc) * sres
                yg = []
                for dt in range(NDT):
                    sl = slice(b * L, (b + 1) * L)
                    t1 = work.tile([128, L], BF16, name="t1")
                    nc.vector.scalar_tensor_tensor(
                        t1[:], xc_t[dt][:, sl], d_t[dt][:], y_ps[dt][:],
                        OP.mult, OP.add)
                    ygt = scanp.tile([128, L], BF16, name=f"yg{dt}", bufs=2)
                    nc.vector.tensor_tensor(ygt[:], t1[:], sres_t[dt][:, sl],
                                            OP.mult)
                    yg.append(ygt)
                # out_proj
                for m in range(L // 128):
                    ps = mm.tile([128, CH], F32, name="ps")
                    ps2 = mm.tile([128, CH], F32, name="ps")
                    for dt in range(NDT):
                        nc.tensor.matmul(
                            ps[:], yg[dt][:, m * 128:(m + 1) * 128],
                            wout_t[dt][:, 0:CH],
                            start=(dt == 0), stop=(dt == NDT - 1))
                        nc.tensor.matmul(
                            ps2[:], yg[dt][:, m * 128:(m + 1) * 128],
                            wout_t[dt][:, CH:2 * CH],
                            start=(dt == 0), stop=(dt == NDT - 1))
                    oc = osb_pool.tile([128, DM], BF16, name="oc", bufs=4)
                    nc.vector.tensor_copy(oc[:, 0:CH], ps[:])
                    nc.vector.tensor_copy(oc[:, CH:2 * CH], ps2[:])
                    nc.sync.dma_start(
                        out=out_part[b * L + m * 128:b * L + (m + 1) * 128, :],
                        in_=oc[:])
            # ReduceScatters emitted after both storms so they don't
            # head-of-line-block the pool queue's scan work.
            for b in range(B):
                if no_cc:
                    nc.sync.dma_start(
                        out=out_rs[b * PB:(b + 1) * PB, :],
                        in_=out_part[b * L:b * L + PB, :])
                else:
                    nc.gpsimd.collective_compute(
                        "ReduceScatter", OP.add, replica_groups=groups,
                        ins=[out_part[b * L:(b + 1) * L, :]],
                        outs=[out_rs[b * PB:(b + 1) * PB, :]])
                nc.sync.dma_start(out=out_ext[b * PB:(b + 1) * PB, :],
                                  in_=out_rs[b * PB:(b + 1) * PB, :])
            rctx.close()

    nc.compile()
    return nc


def prep_inputs(inputs, cfg):
    """Host-side sharding/transposition. Returns per-core input maps."""
    B, L, DM, DI = cfg["B"], cfg["L"], cfg["DM"], cfg["DI"]
    NST, RK, K4, NC = cfg["NST"], cfg["RK"], cfg["K4"], cfg["NC"]
    DL = DI // NC
    NDT = DL // 128
    TOK = B * L

    x = np.asarray(inputs["x"], np.float32)
    W_in = np.asarray(inputs["W_in"], np.float32)
    conv_w = np.asarray(inputs["conv_w"], np.float32)
    conv_b = np.asarray(inputs["conv_b"], np.float32)
    W_x = np.asarray(inputs["W_x"], np.float32)
    W_dt = np.asarray(inputs["W_dt"], np.float32)
    b_dt = np.asarray(inputs["b_dt"], np.float32)
    A_log = np.asarray(inputs["A_log"], np.float32)
    D = np.asarray(inputs["D"], np.float32)
    W_out = np.asarray(inputs["W_out"], np.float32)

    xT = np.ascontiguousarray(x.reshape(TOK, DM).T).astype(BF_NP)
    id128 = np.eye(128, dtype=BF_NP)
    in_maps = []
    for c in range(NC):
        sl = slice(c * DL, (c + 1) * DL)
        w_in_sel = np.concatenate([W_in[sl], W_in[DI + c * DL:DI + (c + 1) * DL]], 0)
        cd = np.zeros((NDT * K4 * 128, 128), np.float32)
        for dt in range(NDT):
            for i in range(K4):
                off = (dt * K4 + i) * 128
                np.fill_diagonal(cd[off:off + 128],
                                 conv_w[c * DL + dt * 128:c * DL + (dt + 1) * 128, i])
        in_maps.append({
            "xT": xT,
            "w_in_T": np.ascontiguousarray(w_in_sel.T).astype(BF_NP),
            "convdiag": cd.astype(BF_NP),
            "w_x_T": np.ascontiguousarray(W_x[:, sl].T).astype(BF_NP),
            "w_dt_T": np.ascontiguousarray(W_dt[sl].T).astype(BF_NP),
            "b_dt_col": np.ascontiguousarray(b_dt[sl])[:, None],
            "a_col": np.ascontiguousarray(-np.exp(A_log[sl])),
            "d_col": np.ascontiguousarray(D[sl])[:, None],
            "conv_b_col": np.ascontiguousarray(conv_b[sl])[:, None],
            "w_out_T": np.ascontiguousarray(W_out[:, sl].T).astype(BF_NP),
            "id128": id128,
        })
    return in_maps


_NC_CACHE = {}


def _get_nc(cfg):
    key = tuple(sorted(cfg.items()))
    if key not in _NC_CACHE:
        _NC_CACHE[key] = build_nc(cfg)
    return _NC_CACHE[key]


def run_cfg(inputs, cfg, time_iters=0):
    nc = _get_nc(cfg)
    NC = cfg["NC"]
    in_maps = prep_inputs(inputs, cfg)
    res = run_bass_kernel_spmd(nc, in_maps, list(range(NC)))
    wall_ns = None
    if time_iters:
        times = []
        for _ in range(time_iters):
            t0 = time.perf_counter()
            res = run_bass_kernel_spmd(nc, in_maps, list(range(NC)))
            times.append(time.perf_counter() - t0)
        wall_ns = min(times) * 1e9
    B, L, DM = cfg["B"], cfg["L"], cfg["DM"]
    PB = L // NC
    parts = []
    for b in range(B):
        for c in range(NC):
            parts.append(np.asarray(res.results[c]["out"][b * PB:(b + 1) * PB],
                                    np.float32))
    out = np.concatenate(parts, 0)
    return out.reshape(B, L, DM), wall_ns


def kernel(**inputs):
    out, _ = run_cfg(inputs, CFG_FULL)
    return out.astype(np.float32)
